# revision 41
# baseline (speedup 1.0000x reference)
"""IsoMaxPlus distance head on 8 NeuronCores — hand-written Bass/Tile kernel.

out[n, c] = -|ds| * sqrt(max(2 - 2 * <f_n/|f_n|, p_c/|p_c|>, eps))

Sharding (per the data-parallel hint): feature rows are sharded 8 ways,
prototypes and distance_scale replicated; no collectives needed.

Per-core algorithm (N_loc=2048, D=2048, C=8192):
  1. Stream feature rows [128, D] in natural layout (paired 2-tile DMAs):
     row norms via a Square+accumulate activation pass (ScalarE); raw f is
     cast to bf16 (VectorE), transposed to the contraction layout with an
     SBUF->SBUF xbar DMA-transpose, and repacked to fp8(e4m3) * 16 with a
     scaled-copy activation into the resident fT8 [d-pairs, n] (4 MB).
     Row norms are folded into the epilogue as a per-partition activation
     scale, so features are never normalized explicitly.
  2. Prototype rows stream the same way but normalized (1/||p_c|| VectorE
     multiply) before the bf16 transpose; each 512-row block is then
     repacked to fp8 * 16 as pT8 [d-pairs, 512]. Block prep runs three
     c-blocks ahead of its matmuls (software pipeline); loads ride the
     ACT HWDGE ring while transposes/output writes ride the SP ring, so
     producer-gated DMAs cannot head-of-line-block the input stream.
  3. Matmul: per (c-block, m-tile), accumulate 8 K-tiles of fp8 DoubleRow
     [128,2,128]x[128,2,512] (K=256 per step) into one PSUM bank (fp32) —
     ~1.4x the bf16 PE rate.
  4. Epilogue: one ScalarE activation u = sqrt(psum * scale_m + 2) reading
     PSUM directly (scale_m = -2/(256*||f_n||) folds the fp8 scaling and
     the feature norm), then one VectorE tensor_scalar o = u * (-|ds|);
     fp32 output tiles are written back in 4-m-tile (1 MB) DMAs.

The first two c-blocks' matmuls are interleaved in m-groups matching the
arrival order of the transposed feature quarters, so the PE starts ~70 us
into the kernel. TimelineSim: ~710 us/core (DMA-bound; HBM+xbar traffic
floor is ~560 us). Accuracy vs the fp32 reference: ~2e-3 max relative
error, from the fp8(e4m3) rounding of the scaled operands.

Set USE_FP8 = False for a pure-bf16 variant (~1.05 ms/core, ~1.4e-4 max
relative error).
"""

import functools
import sys

sys.path.insert(0, "/opt/trn_rl_repo")

import numpy as np

import jax
from jax.experimental.shard_map import shard_map
from jax.sharding import Mesh, NamedSharding, PartitionSpec as P

import concourse.bass as bass
import concourse.mybir as mybir
import concourse.tile as tile
from concourse.bass2jax import bass_jit

N_CORES = 8
PART = 128
F32 = mybir.dt.float32
BF16 = mybir.dt.bfloat16
FP8 = mybir.dt.float8e4
AF = mybir.ActivationFunctionType
ALU = mybir.AluOpType

# fp8 (e4m3) matmul with DoubleRow packing: ~1.4x PE throughput over bf16.
# Operands are scaled by SCALE_F8 before the cast to move the normalized
# values out of the fp8 denormal range; the epilogue scale folds 1/SCALE_F8^2.
USE_FP8 = True
SCALE_F8 = 16.0


def build_iso_kernel(tc, out, f, p, ds):
    """Emit the per-core kernel. out: [N_loc, C] f32; f: [N_loc, D] f32;
    p: [C, D] f32; ds: [1] f32. All APs over DRAM tensors."""
    nc = tc.nc
    n_loc, d = f.shape
    c, d2 = p.shape
    assert d == d2 and n_loc % PART == 0 and d % PART == 0
    kt = d // PART            # K tiles along contraction
    mt = n_loc // PART        # M tiles (feature rows)
    cb = min(512, c)          # c-block width (one PSUM bank)
    ncb = c // cb
    cj = cb // PART           # 128-row chunks per c-block
    fq = min(512, n_loc)      # feature-phase quarter (rows)
    nfq = n_loc // fq
    fjq = fq // PART          # 128-row chunks per feature quarter

    use_fp8 = USE_FP8 and kt % 2 == 0
    kt8 = kt // 2 if use_fp8 else kt
    import math
    ob = math.gcd(4, math.gcd(fjq, mt))   # output-write batch (m-tiles)

    import contextlib

    with contextlib.ExitStack() as ctx:
        consts = ctx.enter_context(tc.tile_pool(name="consts", bufs=1))
        nat = ctx.enter_context(tc.tile_pool(name="nat", bufs=3))
        cast = ctx.enter_context(tc.tile_pool(name="cast", bufs=3))
        sqs = ctx.enter_context(tc.tile_pool(name="sqs", bufs=1))
        small = ctx.enter_context(tc.tile_pool(name="small", bufs=8))
        if use_fp8:
            # bf16 transposed tiles are short-lived staging for the fp8 cast
            ftp = ctx.enter_context(tc.tile_pool(name="ftp", bufs=3))
            ft8p = ctx.enter_context(tc.tile_pool(name="ft8p", bufs=1))
            ptp = ctx.enter_context(tc.tile_pool(name="ptp", bufs=1))
            pt8p = ctx.enter_context(tc.tile_pool(name="pt8p", bufs=4))
        else:
            ftp = ctx.enter_context(tc.tile_pool(name="ftp", bufs=1))
            ptp = ctx.enter_context(tc.tile_pool(name="ptp", bufs=3))
        upool = ctx.enter_context(tc.tile_pool(name="upool", bufs=4))
        opool = ctx.enter_context(tc.tile_pool(name="opool", bufs=2))
        pspool = ctx.enter_context(tc.tile_pool(name="pspool", bufs=6, space="PSUM"))

        # ---- constants ----
        ds_b = consts.tile([PART, 1], F32, tag="ds_b")
        ds_bcast = bass.AP(tensor=ds.tensor, offset=ds.offset, ap=[[0, PART], [1, 1]])
        nc.gpsimd.dma_start(out=ds_b, in_=ds_bcast)
        # scale_all[:, m] = -2 / ||f_row||
        scale_all = consts.tile([PART, mt], F32, tag="scale_all")
        bias2 = consts.tile([PART, 1], F32, tag="bias2")
        nc.vector.memset(bias2, 2.0)
        zero = consts.tile([PART, 1], F32, tag="zero")
        nc.vector.memset(zero, 0.0)
        nds = consts.tile([PART, 1], F32, tag="nds")
        nc.vector.tensor_scalar_mul(nds, ds_b, -1.0)
        negds = consts.tile([PART, 1], F32, tag="negds")
        nc.vector.tensor_tensor(negds, ds_b, nds, ALU.min)

        # epilogue scale per m-tile: -2 / (||f|| * SCALE_F8^2 if fp8)
        epi_mul = -2.0 / (SCALE_F8 * SCALE_F8) if use_fp8 else -2.0

        if use_fp8:
            fT8 = ft8p.tile([PART, kt8, 2, n_loc], FP8, tag="fT8")

        fT = None if use_fp8 else ftp.tile([PART, kt, n_loc], BF16, tag="fT")

        def prep_f_quarter(q):
            """Norms + bf16 cast + SBUF->SBUF xbar transpose (+ fp8 repack)
            for f rows [q*fq, (q+1)*fq)."""
            fnats = []
            for jj2 in range(0, fjq, 2):
                i = q * fjq + jj2
                fnat2 = nat.tile([PART, 2, d], F32, tag="nat", name="nat")
                eng = nc.scalar if jj2 % 4 == 0 else nc.sync
                eng.dma_start(
                    out=fnat2,
                    in_=f[i * PART:(i + 2) * PART, :].rearrange(
                        "(j p) d -> p j d", p=PART),
                )
                fnats += [fnat2[:, 0, :], fnat2[:, 1, :]]
            for jj in range(fjq):
                i = q * fjq + jj
                fnat = fnats[jj]
                # sum of squares on ACT (Square + free-dim accumulate)
                ss = small.tile([PART, 1], F32, tag="small")
                sq = sqs.tile([PART, d], F32, tag="sq")
                nc.scalar.activation(out=sq, in_=fnat, func=AF.Square, accum_out=ss)
                fc = cast.tile([PART, d], BF16, tag="cast")
                nc.vector.tensor_copy(out=fc, in_=fnat)
                if use_fp8:
                    # transpose to a per-tile staging tile, then scaled fp8
                    # repack into the resident fT8 [p, k8, ko, n]
                    fTt = ftp.tile([PART, kt, PART], BF16, tag="fTt", name="fTt")
                    nc.sync.dma_start_transpose(fTt, fc)
                    nc.scalar.activation(
                        out=fT8[:, :, :, i * PART:(i + 1) * PART],
                        in_=fTt[:].rearrange("p (k8 ko) n -> p k8 ko n", ko=2),
                        func=AF.Copy, scale=SCALE_F8,
                    )
                else:
                    # fT[p, k, n=i*128+r] = fc[r, k*128+p]  (d = k*128 + p)
                    nc.sync.dma_start_transpose(fT[:, :, i * PART:(i + 1) * PART], fc)
                nrm = small.tile([PART, 1], F32, tag="small")
                nc.scalar.activation(out=nrm, in_=ss, func=AF.Sqrt, bias=zero)
                inv = small.tile([PART, 1], F32, tag="small")
                nc.vector.reciprocal(inv, nrm)
                nc.vector.tensor_scalar(
                    out=scale_all[:, i:i + 1], in0=inv, scalar1=epi_mul, scalar2=None,
                    op0=ALU.mult,
                )

        def prep_p_block(b, startup=False):
            """Normalize + SBUF->SBUF xbar transpose (+ fp8 repack) for
            prototype rows [b*cb, (b+1)*cb). Returns the matmul rhs tile."""
            pT = ptp.tile([PART, kt, cb], BF16, tag="pT")
            pnats = []
            for j2 in range(0, cj, 2):
                r0 = b * cb + j2 * PART
                pnat2 = nat.tile([PART, 2, d], F32, tag="nat", name="nat")
                eng = nc.sync if (startup and j2 % 4 == 2) else nc.scalar
                eng.dma_start(
                    out=pnat2,
                    in_=p[r0:r0 + 2 * PART, :].rearrange(
                        "(j p) d -> p j d", p=PART),
                )
                pnats += [pnat2[:, 0, :], pnat2[:, 1, :]]
            for j in range(cj):
                pnat = pnats[j]
                ssp = small.tile([PART, 1], F32, tag="small")
                sqp = sqs.tile([PART, d], F32, tag="sq")
                nc.scalar.activation(out=sqp, in_=pnat, func=AF.Square, accum_out=ssp)
                nrmp = small.tile([PART, 1], F32, tag="small")
                nc.scalar.activation(out=nrmp, in_=ssp, func=AF.Sqrt, bias=zero)
                invp = small.tile([PART, 1], F32, tag="small")
                nc.vector.reciprocal(invp, nrmp)
                # normalize + bf16 cast in one DVE op: pc = pnat * (1/||p||)
                pc = cast.tile([PART, d], BF16, tag="cast")
                nc.vector.tensor_scalar_mul(pc, pnat, invp)
                nc.sync.dma_start_transpose(pT[:, :, j * PART:(j + 1) * PART], pc)
            if not use_fp8:
                return pT
            pT8 = pt8p.tile([PART, kt8, 2, cb], FP8, tag="pT8")
            nc.vector.tensor_scalar_mul(
                pT8[:], pT[:].rearrange("p (k8 ko) n -> p k8 ko n", ko=2), SCALE_F8,
            )
            return pT8

        # ---- emission order: first-needed data first ----
        pT_blocks = {}
        prep_f_quarter(0)
        pT_blocks[0] = prep_p_block(0, startup=True)
        if ncb > 1:
            pT_blocks[1] = prep_p_block(1, startup=True)
        for q in range(1, nfq):
            prep_f_quarter(q)

        def matmul_group(b, pT, m_list):
            oo2 = None
            for m in m_list:
                ps = pspool.tile([PART, cb], F32, tag="ps")
                if use_fp8:
                    for k8 in range(kt8):
                        nc.tensor.matmul(
                            ps,
                            lhsT=fT8[:, k8, :, m * PART:(m + 1) * PART],
                            rhs=pT[:, k8, :, :],
                            start=(k8 == 0),
                            stop=(k8 == kt8 - 1),
                            perf_mode=mybir.MatmulPerfMode.DoubleRow,
                        )
                else:
                    for k in range(kt):
                        nc.tensor.matmul(
                            ps,
                            lhsT=fT[:, k, m * PART:(m + 1) * PART],
                            rhs=pT[:, k, :],
                            start=(k == 0),
                            stop=(k == kt - 1),
                        )
                u = upool.tile([PART, cb], F32, tag="u")
                nc.scalar.activation(
                    out=u, in_=ps, func=AF.Sqrt,
                    bias=bias2, scale=scale_all[:, m:m + 1],
                )
                if oo2 is None:
                    oo2 = opool.tile([PART, ob, cb], F32, tag="o", name="oo2")
                    m0 = m
                nc.vector.tensor_scalar_mul(oo2[:, m - m0, :], u, negds)
                if m == m0 + ob - 1:
                    nc.sync.dma_start(
                        out=out[m0 * PART:(m0 + ob) * PART,
                                b * cb:(b + 1) * cb].rearrange(
                                    "(j p) c -> p j c", p=PART),
                        in_=oo2,
                    )
                    oo2 = None

        # Phase 1: first two c-blocks interleaved in m-groups of 4, matching
        # the arrival order of the f quarters (q0, pT0, pT1, q1, q2, q3).
        first = list(range(min(2, ncb)))
        mg = max(1, fjq)
        for g in range(0, mt, mg):
            for b in first:
                matmul_group(b, pT_blocks[b], list(range(g, min(g + mg, mt))))
            if g == 0 and 2 < ncb:
                pT_blocks[2] = prep_p_block(2)
            if g == mg and 3 < ncb:
                pT_blocks[3] = prep_p_block(3)
            if g == 2 * mg and 4 < ncb:
                pT_blocks[4] = prep_p_block(4)
        for b in first:
            pT_blocks.pop(b)

        # Phase 2: steady state with prefetch depth 2.
        for b in range(len(first), ncb):
            if b + 3 < ncb:
                pT_blocks[b + 3] = prep_p_block(b + 3)
            matmul_group(b, pT_blocks.pop(b), list(range(mt)))


@bass_jit
def _iso_bass(nc, f, p, ds):
    out = nc.dram_tensor(
        "out", [f.shape[0], p.shape[0]], F32, kind="ExternalOutput"
    )
    with tile.TileContext(nc) as tc:
        build_iso_kernel(tc, out[:], f[:], p[:], ds[:])
    return out


@functools.cache
def _jitted():
    devices = jax.devices()[:N_CORES]
    mesh = Mesh(np.asarray(devices), ("core",))
    fn = jax.jit(
        shard_map(
            _iso_bass,
            mesh=mesh,
            in_specs=(P("core"), P(), P()),
            out_specs=P("core"),
            check_rep=False,
        )
    )
    return fn, mesh


def kernel(features, prototypes, distance_scale):
    features = np.ascontiguousarray(features, dtype=np.float32)
    prototypes = np.ascontiguousarray(prototypes, dtype=np.float32)
    distance_scale = np.ascontiguousarray(distance_scale, dtype=np.float32)
    fn, mesh = _jitted()
    f = jax.device_put(features, NamedSharding(mesh, P("core")))
    p = jax.device_put(prototypes, NamedSharding(mesh, P()))
    ds = jax.device_put(distance_scale, NamedSharding(mesh, P()))
    out = fn(f, p, ds)
    return np.asarray(jax.device_get(out)).astype(np.float32)


# revision 46
# speedup vs baseline: 1.3609x; 1.3609x over previous
"""IsoMaxPlus distance head on 8 NeuronCores — hand-written Bass/Tile kernel.

out[n, c] = -|ds| * sqrt(max(2 - 2 * <f_n/|f_n|, p_c/|p_c|>, eps))

Sharding (per the data-parallel hint): feature rows are sharded 8 ways,
prototypes and distance_scale replicated; no collectives needed.

Per-core algorithm (N_loc=2048, D=2048, C=8192):
  1. Stream feature rows [128, D] in natural layout (paired 2-tile DMAs):
     row norms via a Square+accumulate activation pass (ScalarE); raw f is
     cast to bf16 (VectorE), transposed to the contraction layout with an
     SBUF->SBUF xbar DMA-transpose, and repacked to fp8(e4m3) * 16 with a
     scaled-copy activation into the resident fT8 [d-pairs, n] (4 MB).
     Row norms are folded into the epilogue as a per-partition activation
     scale, so features are never normalized explicitly.
  2. Prototype rows stream the same way but normalized (1/||p_c|| VectorE
     multiply) before the bf16 transpose; each 512-row block is then
     repacked to fp8 * 16 as pT8 [d-pairs, 512]. Block prep runs three
     c-blocks ahead of its matmuls (software pipeline); loads ride the
     ACT HWDGE ring while transposes/output writes ride the SP ring, so
     producer-gated DMAs cannot head-of-line-block the input stream.
  3. Matmul: per (c-block, m-tile), accumulate 8 K-tiles of fp8 DoubleRow
     [128,2,128]x[128,2,512] (K=256 per step) into one PSUM bank (fp32) —
     ~1.4x the bf16 PE rate.
  4. Epilogue: one ScalarE activation u = sqrt(psum * scale_m + 2) reading
     PSUM directly (scale_m = -2/(256*||f_n||) folds the fp8 scaling and
     the feature norm), then one VectorE tensor_scalar o = u * (-|ds|);
     fp32 output tiles are written back in 4-m-tile (1 MB) DMAs.

The first two c-blocks' matmuls are interleaved in m-groups matching the
arrival order of the transposed feature quarters, so the PE starts ~70 us
into the kernel. TimelineSim: ~700 us/core (DMA-bound; HBM+xbar traffic
floor is ~560 us). Accuracy vs the fp32 reference: ~2e-3 max relative
error, from the fp8(e4m3) rounding of the scaled operands.

Set USE_FP8 = False for a pure-bf16 variant (~1.05 ms/core, ~1.4e-4 max
relative error).
"""

import functools
import sys

sys.path.insert(0, "/opt/trn_rl_repo")

import numpy as np

import jax
from jax.experimental.shard_map import shard_map
from jax.sharding import Mesh, NamedSharding, PartitionSpec as P

import concourse.bass as bass
import concourse.mybir as mybir
import concourse.tile as tile
from concourse.bass2jax import bass_jit

N_CORES = 8
PART = 128
F32 = mybir.dt.float32
BF16 = mybir.dt.bfloat16
FP8 = mybir.dt.float8e4
AF = mybir.ActivationFunctionType
ALU = mybir.AluOpType

# fp8 (e4m3) matmul with DoubleRow packing: ~1.4x PE throughput over bf16.
# Operands are scaled by SCALE_F8 before the cast to move the normalized
# values out of the fp8 denormal range; the epilogue scale folds 1/SCALE_F8^2.
USE_FP8 = True
SCALE_F8 = 16.0


def build_iso_kernel(tc, out, f, p, ds):
    """Emit the per-core kernel. out: [N_loc, C] f32; f: [N_loc, D] f32;
    p: [C, D] f32; ds: [1] f32. All APs over DRAM tensors."""
    nc = tc.nc
    n_loc, d = f.shape
    c, d2 = p.shape
    assert d == d2 and n_loc % PART == 0 and d % PART == 0
    kt = d // PART            # K tiles along contraction
    mt = n_loc // PART        # M tiles (feature rows)
    cb = min(512, c)          # c-block width (one PSUM bank)
    ncb = c // cb
    cj = cb // PART           # 128-row chunks per c-block
    fq = min(512, n_loc)      # feature-phase quarter (rows)
    nfq = n_loc // fq
    fjq = fq // PART          # 128-row chunks per feature quarter

    use_fp8 = USE_FP8 and kt % 2 == 0
    kt8 = kt // 2 if use_fp8 else kt
    import math
    ob = math.gcd(4, math.gcd(fjq, mt))   # output-write batch (m-tiles)

    import contextlib

    with contextlib.ExitStack() as ctx:
        consts = ctx.enter_context(tc.tile_pool(name="consts", bufs=1))
        nat = ctx.enter_context(tc.tile_pool(name="nat", bufs=3))
        cast = ctx.enter_context(tc.tile_pool(name="cast", bufs=3))
        sqs = ctx.enter_context(tc.tile_pool(name="sqs", bufs=1))
        small = ctx.enter_context(tc.tile_pool(name="small", bufs=8))
        if use_fp8:
            # bf16 transposed tiles are short-lived staging for the fp8 cast
            ftp = ctx.enter_context(tc.tile_pool(name="ftp", bufs=3))
            ft8p = ctx.enter_context(tc.tile_pool(name="ft8p", bufs=1))
            ptp = ctx.enter_context(tc.tile_pool(name="ptp", bufs=1))
            pt8p = ctx.enter_context(tc.tile_pool(name="pt8p", bufs=4))
        else:
            ftp = ctx.enter_context(tc.tile_pool(name="ftp", bufs=1))
            ptp = ctx.enter_context(tc.tile_pool(name="ptp", bufs=3))
        upool = ctx.enter_context(tc.tile_pool(name="upool", bufs=4))
        opool = ctx.enter_context(tc.tile_pool(name="opool", bufs=2))
        pspool = ctx.enter_context(tc.tile_pool(name="pspool", bufs=8, space="PSUM"))

        # ---- constants ----
        ds_b = consts.tile([PART, 1], F32, tag="ds_b")
        ds_bcast = bass.AP(tensor=ds.tensor, offset=ds.offset, ap=[[0, PART], [1, 1]])
        nc.gpsimd.dma_start(out=ds_b, in_=ds_bcast)
        # scale_all[:, m] = -2 / ||f_row||
        scale_all = consts.tile([PART, mt], F32, tag="scale_all")
        bias2 = consts.tile([PART, 1], F32, tag="bias2")
        nc.vector.memset(bias2, 2.0)
        zero = consts.tile([PART, 1], F32, tag="zero")
        nc.vector.memset(zero, 0.0)
        nds = consts.tile([PART, 1], F32, tag="nds")
        nc.vector.tensor_scalar_mul(nds, ds_b, -1.0)
        negds = consts.tile([PART, 1], F32, tag="negds")
        nc.vector.tensor_tensor(negds, ds_b, nds, ALU.min)

        # epilogue scale per m-tile: -2 / (||f|| * SCALE_F8^2 if fp8)
        epi_mul = -2.0 / (SCALE_F8 * SCALE_F8) if use_fp8 else -2.0

        if use_fp8:
            fT8 = ft8p.tile([PART, kt8, 2, n_loc], FP8, tag="fT8")

        fT = None if use_fp8 else ftp.tile([PART, kt, n_loc], BF16, tag="fT")

        def prep_f_quarter(q):
            """Norms + bf16 cast + SBUF->SBUF xbar transpose (+ fp8 repack)
            for f rows [q*fq, (q+1)*fq)."""
            fnats = []
            for jj2 in range(0, fjq, 2):
                i = q * fjq + jj2
                fnat2 = nat.tile([PART, 2, d], F32, tag="nat", name="nat")
                eng = nc.scalar if jj2 % 4 == 0 else nc.sync
                eng.dma_start(
                    out=fnat2,
                    in_=f[i * PART:(i + 2) * PART, :].rearrange(
                        "(j p) d -> p j d", p=PART),
                )
                fnats += [fnat2[:, 0, :], fnat2[:, 1, :]]
            for jj in range(fjq):
                i = q * fjq + jj
                fnat = fnats[jj]
                # sum of squares on ACT (Square + free-dim accumulate)
                ss = small.tile([PART, 1], F32, tag="small")
                sq = sqs.tile([PART, d], F32, tag="sq")
                nc.scalar.activation(out=sq, in_=fnat, func=AF.Square, accum_out=ss)
                fc = cast.tile([PART, d], BF16, tag="cast")
                nc.vector.tensor_copy(out=fc, in_=fnat)
                if use_fp8:
                    # transpose to a per-tile staging tile, then scaled fp8
                    # repack into the resident fT8 [p, k8, ko, n]
                    fTt = ftp.tile([PART, kt, PART], BF16, tag="fTt", name="fTt")
                    nc.sync.dma_start_transpose(fTt, fc)
                    nc.scalar.activation(
                        out=fT8[:, :, :, i * PART:(i + 1) * PART],
                        in_=fTt[:].rearrange("p (k8 ko) n -> p k8 ko n", ko=2),
                        func=AF.Copy, scale=SCALE_F8,
                    )
                else:
                    # fT[p, k, n=i*128+r] = fc[r, k*128+p]  (d = k*128 + p)
                    nc.sync.dma_start_transpose(fT[:, :, i * PART:(i + 1) * PART], fc)
                nrm = small.tile([PART, 1], F32, tag="small")
                nc.scalar.activation(out=nrm, in_=ss, func=AF.Sqrt, bias=zero)
                inv = small.tile([PART, 1], F32, tag="small")
                nc.vector.reciprocal(inv, nrm)
                nc.vector.tensor_scalar(
                    out=scale_all[:, i:i + 1], in0=inv, scalar1=epi_mul, scalar2=None,
                    op0=ALU.mult,
                )

        def prep_p_block(b, startup=False):
            """Normalize + SBUF->SBUF xbar transpose (+ fp8 repack) for
            prototype rows [b*cb, (b+1)*cb). Returns the matmul rhs tile."""
            pT = ptp.tile([PART, kt, cb], BF16, tag="pT")
            pnats = []
            for j2 in range(0, cj, 2):
                r0 = b * cb + j2 * PART
                pnat2 = nat.tile([PART, 2, d], F32, tag="nat", name="nat")
                eng = nc.sync if (startup and j2 % 4 == 2) else nc.scalar
                eng.dma_start(
                    out=pnat2,
                    in_=p[r0:r0 + 2 * PART, :].rearrange(
                        "(j p) d -> p j d", p=PART),
                )
                pnats += [pnat2[:, 0, :], pnat2[:, 1, :]]
            for j in range(cj):
                pnat = pnats[j]
                ssp = small.tile([PART, 1], F32, tag="small")
                sqp = sqs.tile([PART, d], F32, tag="sq")
                nc.scalar.activation(out=sqp, in_=pnat, func=AF.Square, accum_out=ssp)
                nrmp = small.tile([PART, 1], F32, tag="small")
                nc.scalar.activation(out=nrmp, in_=ssp, func=AF.Sqrt, bias=zero)
                invp = small.tile([PART, 1], F32, tag="small")
                nc.vector.reciprocal(invp, nrmp)
                # normalize + bf16 cast in one DVE op: pc = pnat * (1/||p||)
                pc = cast.tile([PART, d], BF16, tag="cast")
                nc.vector.tensor_scalar_mul(pc, pnat, invp)
                nc.sync.dma_start_transpose(pT[:, :, j * PART:(j + 1) * PART], pc)
            if not use_fp8:
                return pT
            pT8 = pt8p.tile([PART, kt8, 2, cb], FP8, tag="pT8")
            nc.vector.tensor_scalar_mul(
                pT8[:], pT[:].rearrange("p (k8 ko) n -> p k8 ko n", ko=2), SCALE_F8,
            )
            return pT8

        # ---- emission order: first-needed data first ----
        pT_blocks = {}
        prep_f_quarter(0)
        pT_blocks[0] = prep_p_block(0, startup=True)
        if ncb > 1:
            pT_blocks[1] = prep_p_block(1, startup=True)
        for q in range(1, nfq):
            prep_f_quarter(q)

        def matmul_group(b, pT, m_list):
            oo2 = None
            for m in m_list:
                ps = pspool.tile([PART, cb], F32, tag="ps")
                if use_fp8:
                    for k8 in range(kt8):
                        nc.tensor.matmul(
                            ps,
                            lhsT=fT8[:, k8, :, m * PART:(m + 1) * PART],
                            rhs=pT[:, k8, :, :],
                            start=(k8 == 0),
                            stop=(k8 == kt8 - 1),
                            perf_mode=mybir.MatmulPerfMode.DoubleRow,
                        )
                else:
                    for k in range(kt):
                        nc.tensor.matmul(
                            ps,
                            lhsT=fT[:, k, m * PART:(m + 1) * PART],
                            rhs=pT[:, k, :],
                            start=(k == 0),
                            stop=(k == kt - 1),
                        )
                u = upool.tile([PART, cb], F32, tag="u")
                nc.scalar.activation(
                    out=u, in_=ps, func=AF.Sqrt,
                    bias=bias2, scale=scale_all[:, m:m + 1],
                )
                if oo2 is None:
                    oo2 = opool.tile([PART, ob, cb], F32, tag="o", name="oo2")
                    m0 = m
                nc.vector.tensor_scalar_mul(oo2[:, m - m0, :], u, negds)
                if m == m0 + ob - 1:
                    nc.sync.dma_start(
                        out=out[m0 * PART:(m0 + ob) * PART,
                                b * cb:(b + 1) * cb].rearrange(
                                    "(j p) c -> p j c", p=PART),
                        in_=oo2,
                    )
                    oo2 = None

        # Phase 1: first two c-blocks interleaved in m-groups of 4, matching
        # the arrival order of the f quarters (q0, pT0, pT1, q1, q2, q3).
        first = list(range(min(2, ncb)))
        mg = max(1, fjq)
        for g in range(0, mt, mg):
            for b in first:
                matmul_group(b, pT_blocks[b], list(range(g, min(g + mg, mt))))
            if g == 0 and 2 < ncb:
                pT_blocks[2] = prep_p_block(2)
            if g == mg and 3 < ncb:
                pT_blocks[3] = prep_p_block(3)
            if g == 2 * mg and 4 < ncb:
                pT_blocks[4] = prep_p_block(4)
        for b in first:
            pT_blocks.pop(b)

        # Phase 2: steady state with prefetch depth 2.
        for b in range(len(first), ncb):
            if b + 3 < ncb:
                pT_blocks[b + 3] = prep_p_block(b + 3)
            matmul_group(b, pT_blocks.pop(b), list(range(mt)))


@bass_jit
def _iso_bass(nc, f, p, ds):
    out = nc.dram_tensor(
        "out", [f.shape[0], p.shape[0]], F32, kind="ExternalOutput"
    )
    with tile.TileContext(nc) as tc:
        build_iso_kernel(tc, out[:], f[:], p[:], ds[:])
    return out


@functools.cache
def _jitted():
    devices = jax.devices()[:N_CORES]
    mesh = Mesh(np.asarray(devices), ("core",))
    fn = jax.jit(
        shard_map(
            _iso_bass,
            mesh=mesh,
            in_specs=(P("core"), P(), P()),
            out_specs=P("core"),
            check_rep=False,
        )
    )
    return fn, mesh


def kernel(features, prototypes, distance_scale):
    features = np.ascontiguousarray(features, dtype=np.float32)
    prototypes = np.ascontiguousarray(prototypes, dtype=np.float32)
    distance_scale = np.ascontiguousarray(distance_scale, dtype=np.float32)
    fn, mesh = _jitted()
    f = jax.device_put(features, NamedSharding(mesh, P("core")))
    p = jax.device_put(prototypes, NamedSharding(mesh, P()))
    ds = jax.device_put(distance_scale, NamedSharding(mesh, P()))
    out = fn(f, p, ds)
    return np.asarray(jax.device_get(out)).astype(np.float32)
